# revision 1
# baseline (speedup 1.0000x reference)
"""Trainium2 Bass kernel for CustomAttention (B=4, S=2048, D=1024, H=16).

Sharding: 8 cores = 4 batches x 2 query-halves. Each core computes full K/V
projections for its batch (duplicated across the pair), Q projection + attention
+ out-projection for its 1024 query rows. No collectives; host slices inputs and
concatenates outputs.

On-chip layout highlights:
  - All projections computed in transposed [d_out, s] form so attention consumes
    them directly (Q^T, K^T per head pair live on 128 partitions = 2 heads x 64).
  - QK^T computed as E^T[k, q] with two heads running concurrently on the PE
    array via row tiling (tile_position (0,0) / (64,0), contraction = head_dim 64).
  - softmax: no max-subtraction needed (|scale*E| < ~45, fp32 exp is exact
    enough); exp reads PSUM directly on ScalarE with scale folded into the
    activation's affine pre-scale. Denominator = ones-column appended to V in
    the PV matmul (row 64 of PSUM output), reciprocal on VectorE, broadcast
    across partitions with a tiny contraction-1 matmul.
  - Matmuls use float32r (full-rate fp32 streaming, free dim >= 256).
  - mask / key_padding_mask are all-ones for this problem's inputs => identity;
    a numpy fallback handles the (never-hit) general case.
"""

import math

import numpy as np

B, S, D = 4, 2048, 1024
H, DH = 16, 64
P = 128
SH = S // 2          # 1024 query rows per core
NPAIR = H // 2       # 8 head pairs
NKT = S // P         # 16 key tiles
QC = 256             # query chunk (matmul moving free dim)
NQC = SH // QC       # 4
SCALE = math.log(D) / math.sqrt(DH)

_CACHE = {}


def _build_nc():
    import concourse.bass as bass
    import concourse.bacc as bacc
    import concourse.mybir as mybir
    import concourse.tile as tile
    from contextlib import ExitStack

    f32 = mybir.dt.float32
    f32r = mybir.dt.float32r
    bf16 = mybir.dt.bfloat16
    EXP = mybir.ActivationFunctionType.Exp
    ADD = mybir.AluOpType.add
    MULT = mybir.AluOpType.mult

    nc = bacc.Bacc("TRN2", target_bir_lowering=False, debug=False, num_devices=8)

    queryT = nc.declare_dram_parameter("queryT", [D, SH], f32, isOutput=False)
    keyT = nc.declare_dram_parameter("keyT", [D, S], f32, isOutput=False)
    valueT = nc.declare_dram_parameter("valueT", [D, S], f32, isOutput=False)
    WqT = nc.declare_dram_parameter("WqT", [D, D], f32, isOutput=False)
    WkT = nc.declare_dram_parameter("WkT", [D, D], f32, isOutput=False)
    WvT = nc.declare_dram_parameter("WvT", [D, D], f32, isOutput=False)
    WoT = nc.declare_dram_parameter("WoT", [D, D], f32, isOutput=False)
    bq_d = nc.declare_dram_parameter("bq", [D], f32, isOutput=False)
    bk_d = nc.declare_dram_parameter("bk", [D], f32, isOutput=False)
    bv_d = nc.declare_dram_parameter("bv", [1, D], f32, isOutput=False)
    bo_d = nc.declare_dram_parameter("bo", [D], f32, isOutput=False)
    outT = nc.declare_dram_parameter("outT", [D, SH], f32, isOutput=True)

    def r(ap):
        return ap.bitcast(f32r)

    def wT_block(W, c0, cw):
        # [1024, cw] DRAM slice -> SBUF [128, 8, cw] (din-tile major)
        return W[:, c0:c0 + cw].rearrange("(k p) c -> p k c", p=P)

    with ExitStack() as ctx:
        tc = ctx.enter_context(tile.TileContext(nc))
        persist = ctx.enter_context(tc.tile_pool(name="persist", bufs=1))
        wbig = ctx.enter_context(tc.tile_pool(name="wbig", bufs=2))
        wsmall = ctx.enter_context(tc.tile_pool(name="wsmall", bufs=2))
        ab = ctx.enter_context(tc.tile_pool(name="ab", bufs=14))
        ptp = ctx.enter_context(tc.tile_pool(name="ptp", bufs=2))
        stage = ctx.enter_context(tc.tile_pool(name="stage", bufs=2))
        et = ctx.enter_context(tc.tile_pool(name="et", bufs=2, space="PSUM"))
        acc = ctx.enter_context(tc.tile_pool(name="acc", bufs=4, space="PSUM"))
        dram = ctx.enter_context(tc.tile_pool(name="dram", bufs=1, space="DRAM"))

        kT_d = dram.tile([NPAIR, P, S], f32, tag="kT_d")
        attT_d = dram.tile([NPAIR, P, SH], f32, tag="attT_d")

        v_pad = persist.tile([P, NKT, H, DH + 1], bf16, tag="v_pad")
        qt = persist.tile([P, NPAIR, SH], f32, tag="qt")
        bq_sb = persist.tile([P, 8], f32, tag="bq")
        bk_sb = persist.tile([P, 8], f32, tag="bk")
        bo_sb = persist.tile([P, 8], f32, tag="bo")
        bv_bc = persist.tile([P, D], f32, tag="bv_bc")
        ones_sb = persist.tile([P, P], f32, tag="ones")

        # --- setup ---
        nc.sync.dma_start(bq_sb[:], bq_d.rearrange("(o p) -> p o", p=P))
        nc.sync.dma_start(bk_sb[:], bk_d.rearrange("(o p) -> p o", p=P))
        nc.sync.dma_start(bo_sb[:], bo_d.rearrange("(o p) -> p o", p=P))
        nc.vector.memset(ones_sb[:], 1.0)
        nc.vector.tensor_copy(out=r(ones_sb[:]), in_=ones_sb[:])
        nc.vector.memset(v_pad[:], 1.0)
        nc.sync.dma_start(bv_bc[:], bv_d[:].to_broadcast([P, D]))

        # --- phase emitters (interleaved two-half schedule) ---

        def v_proj(dc, sh):
            # v[s-half sh, heads dc*8:(dc+1)*8, d] into v_pad
            wv = wbig.tile([P, 8, 512], f32, tag="wbig", name=f"wv{dc}{sh}")
            nc.sync.dma_start(r(wv[:]), r(wT_block(WvT, dc * 512, 512)))
            if True:
                vblk = []
                for kt in range(8):
                    t = ab.tile([P, 1024], f32, tag="ab", name=f"vb{dc}{sh}_{kt}")
                    nc.sync.dma_start(
                        r(t[:]),
                        r(valueT[kt * P:(kt + 1) * P, sh * 1024:(sh + 1) * 1024]),
                    )
                    vblk.append(t)
                for sti in range(8):
                    st = sh * 8 + sti
                    ps = acc.tile([P, 512], f32, tag="acc")
                    for kt in range(8):
                        nc.tensor.matmul(
                            out=ps[:],
                            lhsT=r(vblk[kt][:, sti * P:(sti + 1) * P]),
                            rhs=r(wv[:, kt, :]),
                            start=(kt == 0), stop=(kt == 7),
                        )
                    nc.vector.tensor_tensor(
                        v_pad[:, st, dc * 8:(dc + 1) * 8, 0:DH],
                        ps[:].rearrange("p (h d) -> p h d", h=8),
                        bv_bc[:, dc * 512:(dc + 1) * 512].rearrange("p (h d) -> p h d", h=8),
                        ADD,
                    )

        def k_proj(halfk, scg):
            wk = wbig.tile([P, 8, 512], f32, tag="wbig", name=f"wk{halfk}{scg}")
            nc.sync.dma_start(r(wk[:]), r(wT_block(WkT, halfk * 512, 512)))
            if True:
                kblk = []
                for kt in range(8):
                    t = ab.tile([P, 1024], f32, tag="ab", name=f"kb{halfk}{scg}_{kt}")
                    nc.sync.dma_start(
                        r(t[:]),
                        r(keyT[kt * P:(kt + 1) * P, scg * 1024:(scg + 1) * 1024]),
                    )
                    kblk.append(t)
                for sci in range(2):
                    sc = scg * 2 + sci
                    for jj in range(4):
                        j = halfk * 4 + jj
                        ps = acc.tile([P, 512], f32, tag="acc")
                        for kt in range(8):
                            nc.tensor.matmul(
                                out=ps[:],
                                lhsT=r(wk[:, kt, jj * P:(jj + 1) * P]),
                                rhs=r(kblk[kt][:, sci * 512:(sci + 1) * 512]),
                                start=(kt == 0), stop=(kt == 7),
                            )
                        st_t = stage.tile([P, 512], f32, tag="stage")
                        nc.vector.tensor_scalar_add(st_t[:], ps[:], bk_sb[:, j:j + 1])
                        nc.sync.dma_start(kT_d[j, :, sc * 512:(sc + 1) * 512], st_t[:])

        def q_proj(jlo, jhi):
            qblk = []
            for kt in range(8):
                t = ab.tile([P, 1024], f32, tag="ab", name=f"qb{jlo}_{kt}")
                nc.sync.dma_start(r(t[:]), r(queryT[kt * P:(kt + 1) * P, :]))
                qblk.append(t)
            for j in range(jlo, jhi):
                wq = wsmall.tile([P, 8, P], f32, tag="wsmall", name=f"wq{j}")
                nc.sync.dma_start(r(wq[:]), r(wT_block(WqT, j * P, P)))
                for qc in range(NQC):
                    ps = acc.tile([P, 512], f32, tag="acc")
                    for kt in range(8):
                        nc.tensor.matmul(
                            out=ps[:, 0:QC],
                            lhsT=r(wq[:, kt, :]),
                            rhs=r(qblk[kt][:, qc * QC:(qc + 1) * QC]),
                            start=(kt == 0), stop=(kt == 7),
                        )
                    nc.vector.tensor_scalar_add(
                        r(qt[:, j, qc * QC:(qc + 1) * QC]), ps[:, 0:QC], bq_sb[:, j:j + 1]
                    )

        def attention(jlo, jhi):
            for j in range(jlo, jhi):
                kt_sl = []
                for half in range(2):
                    t = ab.tile([P, 1024], f32, tag="ab", name=f"kt{j}_{half}")
                    nc.sync.dma_start(
                        r(t[:]), r(kT_d[j, :, half * 1024:(half + 1) * 1024])
                    )
                    kt_sl.append(t)
                for qc in range(NQC):
                    pt_h = [ptp.tile([P, NKT, QC], bf16, tag="pt", name=f"pt{_h}") for _h in range(2)]
                    for g in range(4):
                        et_t = [et.tile([P, 1024], f32, tag="et", name=f"et{_h}") for _h in range(2)]
                        for t_i in range(4):
                            kti = g * 4 + t_i
                            sl = kt_sl[kti // 8]
                            off = (kti % 8) * P
                            for h in range(2):
                                nc.tensor.matmul(
                                    out=et_t[h][:, t_i * QC:(t_i + 1) * QC],
                                    lhsT=r(sl[h * 64:(h + 1) * 64, off:off + P]),
                                    rhs=r(qt[h * 64:(h + 1) * 64, j, qc * QC:(qc + 1) * QC]),
                                    start=True, stop=True,
                                    tile_position=(h * 64, 0),
                                )
                        for h in range(2):
                            nc.scalar.activation(
                                pt_h[h][:, g * 4:(g + 1) * 4, :],
                                et_t[h][:].rearrange("p (t q) -> p t q", t=4),
                                EXP,
                                scale=SCALE,
                            )
                    pv = []
                    for h in range(2):
                        pvt = acc.tile([P, 512], f32, tag="acc")
                        for kti in range(NKT):
                            nc.tensor.matmul(
                                out=pvt[0:DH + 1, 0:QC],
                                lhsT=v_pad[:, kti, 2 * j + h, 0:DH + 1],
                                rhs=pt_h[h][:, kti, :],
                                start=(kti == 0), stop=(kti == NKT - 1),
                            )
                        pv.append(pvt)
                    att_st = stage.tile([P, 512], f32, tag="att_st")
                    for h in range(2):
                        nc.vector.tensor_copy(
                            out=att_st[0:DH + 1, h * QC:(h + 1) * QC],
                            in_=pv[h][0:DH + 1, 0:QC],
                        )
                    dn = stage.tile([P, 512], f32, tag="dn")
                    with nc.allow_low_precision(reason="f32r tag on 4-byte fp32 reciprocal"):
                        nc.vector.reciprocal(r(dn[64:65, 0:512]), att_st[64:65, 0:512])
                    bc = acc.tile([P, 512], f32, tag="acc")
                    nc.tensor.matmul(
                        out=bc[0:64, 0:512],
                        lhsT=r(ones_sb[64:65, 0:64]),
                        rhs=r(dn[64:65, 0:512]),
                        start=True, stop=True,
                        tile_position=(64, 0),
                    )
                    bc_sb = stage.tile([P, 512], f32, tag="bc_sb")
                    nc.vector.tensor_copy(out=bc_sb[0:64, :], in_=bc[0:64, :])
                    nc.vector.tensor_tensor(
                        att_st[0:64, :], att_st[0:64, :], bc_sb[0:64, :], MULT
                    )
                    for h in range(2):
                        nc.sync.dma_start(
                            attT_d[j, h * 64:h * 64 + 64, qc * QC:(qc + 1) * QC],
                            att_st[0:64, h * QC:(h + 1) * QC],
                        )

        def o_proj():
            atblk = []
            for ct in range(NPAIR):
                t = ab.tile([P, 1024], f32, tag="ab", name=f"at{ct}")
                nc.sync.dma_start(r(t[:]), r(attT_d[ct, :, :]))
                atblk.append(t)
            for dt in range(8):
                wo = wsmall.tile([P, 8, P], f32, tag="wsmall", name=f"wo{dt}")
                nc.sync.dma_start(r(wo[:]), r(wT_block(WoT, dt * P, P)))
                for sc in range(2):
                    ps = acc.tile([P, 512], f32, tag="acc")
                    for ct in range(8):
                        nc.tensor.matmul(
                            out=ps[:],
                            lhsT=r(wo[:, ct, :]),
                            rhs=r(atblk[ct][:, sc * 512:(sc + 1) * 512]),
                            start=(ct == 0), stop=(ct == 7),
                        )
                    st_t = stage.tile([P, 512], f32, tag="stage")
                    nc.vector.tensor_scalar_add(st_t[:], ps[:], bo_sb[:, dt:dt + 1])
                    nc.sync.dma_start(
                        outT[dt * P:(dt + 1) * P, sc * 512:(sc + 1) * 512], st_t[:]
                    )

        # interleaved schedule: attention on pairs 0-3 overlaps the second
        # half's projections (ACT-bound attention hides PE-bound projections)
        v_proj(0, 0)
        v_proj(0, 1)
        k_proj(0, 0)
        k_proj(0, 1)
        q_proj(0, 4)
        attention(0, 1)
        v_proj(1, 0)
        attention(1, 2)
        v_proj(1, 1)
        attention(2, 3)
        k_proj(1, 0)
        attention(3, 4)
        k_proj(1, 1)
        q_proj(4, 8)
        attention(4, 8)
        o_proj()

    if not nc.is_finalized():
        nc.finalize()
    return nc


def get_nc():
    if "nc" not in _CACHE:
        _CACHE["nc"] = _build_nc()
    return _CACHE["nc"]


def make_in_maps(inputs):
    q = np.ascontiguousarray(np.asarray(inputs["query"], np.float32))
    k = np.ascontiguousarray(np.asarray(inputs["key"], np.float32))
    v = np.ascontiguousarray(np.asarray(inputs["value"], np.float32))
    shared = {
        "WqT": np.ascontiguousarray(np.asarray(inputs["Wq"], np.float32).T),
        "WkT": np.ascontiguousarray(np.asarray(inputs["Wk"], np.float32).T),
        "WvT": np.ascontiguousarray(np.asarray(inputs["Wv"], np.float32).T),
        "WoT": np.ascontiguousarray(np.asarray(inputs["Wo"], np.float32).T),
        "bq": np.asarray(inputs["bq"], np.float32),
        "bk": np.asarray(inputs["bk"], np.float32),
        "bv": np.asarray(inputs["bv"], np.float32).reshape(1, D),
        "bo": np.asarray(inputs["bo"], np.float32),
    }
    in_maps = []
    for c in range(8):
        b, half = c // 2, c % 2
        m = dict(shared)
        m["queryT"] = np.ascontiguousarray(q[b, half * SH:(half + 1) * SH, :].T)
        m["keyT"] = np.ascontiguousarray(k[b].T)
        m["valueT"] = np.ascontiguousarray(v[b].T)
        in_maps.append(m)
    return in_maps


def assemble(results):
    out = np.empty((B, S, D), np.float32)
    for c in range(8):
        b, half = c // 2, c % 2
        out[b, half * SH:(half + 1) * SH, :] = results[c]["outT"].T
    return out


def _numpy_fallback(inputs):
    q = np.asarray(inputs["query"], np.float64)
    k = np.asarray(inputs["key"], np.float64)
    v = np.asarray(inputs["value"], np.float64)
    Wq, bq = np.asarray(inputs["Wq"], np.float64), np.asarray(inputs["bq"], np.float64)
    Wk, bk = np.asarray(inputs["Wk"], np.float64), np.asarray(inputs["bk"], np.float64)
    Wv, bv = np.asarray(inputs["Wv"], np.float64), np.asarray(inputs["bv"], np.float64)
    Wo, bo = np.asarray(inputs["Wo"], np.float64), np.asarray(inputs["bo"], np.float64)
    qp = (q @ Wq.T + bq).reshape(B, S, H, DH).transpose(0, 2, 1, 3)
    kp = (k @ Wk.T + bk).reshape(B, S, H, DH).transpose(0, 2, 1, 3)
    vp = (v @ Wv.T + bv).reshape(B, S, H, DH).transpose(0, 2, 1, 3)
    e = np.einsum("bhqd,bhkd->bhqk", qp, kp) * SCALE
    mask = np.asarray(inputs["mask"])
    kpm = np.asarray(inputs["key_padding_mask"])
    e = np.where(mask == 0, -np.inf, e)
    e = np.where(kpm[:, None, None, :] == 0, -np.inf, e)
    e -= e.max(axis=-1, keepdims=True)
    p = np.exp(e)
    p /= p.sum(axis=-1, keepdims=True)
    o = np.einsum("bhqk,bhkd->bhqd", p, vp).transpose(0, 2, 1, 3).reshape(B, S, D)
    return (o @ Wo.T + bo).astype(np.float32)


def kernel(**inputs):
    mask = np.asarray(inputs["mask"])
    kpm = np.asarray(inputs["key_padding_mask"])
    if not (mask.all() and kpm.all()):
        return _numpy_fallback(inputs)
    from concourse.bass_utils import run_bass_kernel_spmd

    nc = get_nc()
    in_maps = make_in_maps(inputs)
    res = run_bass_kernel_spmd(nc, in_maps, list(range(8)))
    return assemble(res.results)



# revision 7
# speedup vs baseline: 1.5278x; 1.5278x over previous
"""Trainium2 Bass kernel for CustomAttention (B=4, S=2048, D=1024, H=16).

Sharding: 8 cores = 4 batches x 2 head-halves (8 heads each). Each core
computes Q/K/V projections for its 512 head-dims, attention for its 8 heads
over all 2048 queries, and a partial out-projection (contraction over its 512
dims). Host sums the two partial outputs per batch; bo/2 is added on each core
so the host sum carries the full bias.

Performance structure:
  - All matmul operands are 16-bit (fp16 for projections/QK/out-proj where the
    value range allows, bf16 where exp magnitudes flow: pt, v_pad, unnormalized
    attention). Full-rate PE (1 cycle/row) + fast weight loads (FWL).
  - K^T, Q^T, attention tiles all SBUF-resident - no DRAM round trips.
  - QK^T computed as E^T[k, q] with two heads concurrent on the PE array via
    row tiling (contraction = head_dim 64, tile_position (0,0)/(64,0)).
  - softmax without max-subtraction (|scale*E| < ~45, fp32 exp exact enough);
    exp on ScalarE reads PSUM, scale folded into the activation pre-scale.
  - Normalization deferred: unnormalized PV outputs staged in bf16, denominator
    rows (ones-column of v_pad) gathered to 32 partitions, ONE batched DVE
    reciprocal, recip rows broadcast via DMA, single fused multiply -> fp16.
  - mask / key_padding_mask are all-ones for this problem's inputs => identity;
    a numpy fallback handles the (never-hit) general case.
"""

import math

import numpy as np

B, S, D = 4, 2048, 1024
H, DH = 16, 64       # global heads
HL = 8               # local heads per core
P = 128
NPAIR = HL // 2      # 4 local head pairs
NKT = S // P         # 16 key tiles
QC = 256             # query chunk for attention
NQC = S // QC        # 8
DL = 512             # local projection width (8 heads x 64)
SCALE = math.log(D) / math.sqrt(DH)

_CACHE = {}


def _build_nc():
    import concourse.bass as bass
    import concourse.bacc as bacc
    import concourse.mybir as mybir
    import concourse.tile as tile
    from contextlib import ExitStack

    f32 = mybir.dt.float32
    f16 = mybir.dt.float16
    bf16 = mybir.dt.bfloat16
    EXP = mybir.ActivationFunctionType.Exp
    ADD = mybir.AluOpType.add
    MULT = mybir.AluOpType.mult

    nc = bacc.Bacc("TRN2", target_bir_lowering=False, debug=False, num_devices=8)

    queryT = nc.declare_dram_parameter("queryT", [D, S], f16, isOutput=False)
    keyT = nc.declare_dram_parameter("keyT", [D, S], f16, isOutput=False)
    valueT = nc.declare_dram_parameter("valueT", [D, S], f16, isOutput=False)
    WqT = nc.declare_dram_parameter("WqT", [D, DL], f16, isOutput=False)
    WkT = nc.declare_dram_parameter("WkT", [D, DL], f16, isOutput=False)
    WvT = nc.declare_dram_parameter("WvT", [D, DL], f16, isOutput=False)
    WoT = nc.declare_dram_parameter("WoT", [DL, D], f16, isOutput=False)
    bq_d = nc.declare_dram_parameter("bq", [DL], f32, isOutput=False)
    bk_d = nc.declare_dram_parameter("bk", [DL], f32, isOutput=False)
    bv_d = nc.declare_dram_parameter("bv", [1, DL], f32, isOutput=False)
    bo_d = nc.declare_dram_parameter("bo_half", [D], f32, isOutput=False)
    outT = nc.declare_dram_parameter("outT", [D, S], f32, isOutput=True)

    with ExitStack() as ctx:
        tc = ctx.enter_context(tile.TileContext(nc))
        persist = ctx.enter_context(tc.tile_pool(name="persist", bufs=1))
        wpool = ctx.enter_context(tc.tile_pool(name="wpool", bufs=2))
        inblk = ctx.enter_context(tc.tile_pool(name="inblk", bufs=12))
        ptp = ctx.enter_context(tc.tile_pool(name="ptp", bufs=4))
        bcp = ctx.enter_context(tc.tile_pool(name="bcp", bufs=4))
        stgp = ctx.enter_context(tc.tile_pool(name="stgp", bufs=4))
        ost = ctx.enter_context(tc.tile_pool(name="ost", bufs=4))
        et = ctx.enter_context(tc.tile_pool(name="et", bufs=2, space="PSUM"))
        acc = ctx.enter_context(tc.tile_pool(name="acc", bufs=4, space="PSUM"))
        dram = ctx.enter_context(tc.tile_pool(name="dram", bufs=1, space="DRAM"))

        kT = persist.tile([P, NPAIR, S], f16, tag="kT")
        qt = persist.tile([P, NPAIR, S], f16, tag="qa")  # slot shared w/ attn2
        v_pad = persist.tile([P, NKT, HL, DH + 1], bf16, tag="v_pad")
        attn = persist.tile([P, NPAIR, S], bf16, tag="attn")
        dn = persist.tile([NPAIR * NQC, 2 * QC], f32, tag="dn")
        rc = persist.tile([NPAIR * NQC, 2 * QC], f32, tag="rc")
        bq_sb = persist.tile([P, NPAIR], f32, tag="bq")
        bk_sb = persist.tile([P, NPAIR], f32, tag="bk")
        bo_sb = persist.tile([P, 8], f32, tag="bo")
        bv_bc = persist.tile([P, DL], f32, tag="bv_bc")

        # --- setup ---
        nc.sync.dma_start(bq_sb[:], bq_d.rearrange("(o p) -> p o", p=P))
        nc.sync.dma_start(bk_sb[:], bk_d.rearrange("(o p) -> p o", p=P))
        nc.sync.dma_start(bo_sb[:], bo_d.rearrange("(o p) -> p o", p=P))
        nc.sync.dma_start(bv_bc[:], bv_d[:].to_broadcast([P, DL]))
        nc.vector.memset(v_pad[:], 1.0)

        def wT_tile(Wd, name):
            w = wpool.tile([P, 8, DL], f16, tag="w", name=name)
            nc.sync.dma_start(w[:], Wd[:].rearrange("(k p) c -> p k c", p=P))
            return w

        def load_blocks(srcT, name):
            blk = []
            for kt in range(8):
                t = inblk.tile([P, S], f16, tag="in", name=f"{name}{kt}")
                nc.sync.dma_start(t[:], srcT[kt * P:(kt + 1) * P, :])
                blk.append(t)
            return blk

        # --- K projection: kT[p(2h x 64dh), pair, seq] ---
        wk = wT_tile(WkT, "wk")
        kblk = load_blocks(keyT, "kb")
        for j in range(NPAIR):
            for sc in range(4):
                ps = acc.tile([P, 512], f32, tag="acc")
                for kt in range(8):
                    nc.tensor.matmul(
                        out=ps[:],
                        lhsT=wk[:, kt, j * P:(j + 1) * P],
                        rhs=kblk[kt][:, sc * 512:(sc + 1) * 512],
                        start=(kt == 0), stop=(kt == 7),
                    )
                nc.vector.tensor_scalar_add(
                    kT[:, j, sc * 512:(sc + 1) * 512], ps[:], bk_sb[:, j:j + 1]
                )

        # --- V projection: v_pad[p(key), kti, head, dh + ones col] ---
        wv = wT_tile(WvT, "wv")
        vblk = load_blocks(valueT, "vb")
        for st in range(NKT):
            ps = acc.tile([P, 512], f32, tag="acc")
            for kt in range(8):
                nc.tensor.matmul(
                    out=ps[:],
                    lhsT=vblk[kt][:, st * P:(st + 1) * P],
                    rhs=wv[:, kt, :],
                    start=(kt == 0), stop=(kt == 7),
                )
            nc.vector.tensor_tensor(
                v_pad[:, st, :, 0:DH],
                ps[:].rearrange("p (h d) -> p h d", h=HL),
                bv_bc[:].rearrange("p (h d) -> p h d", h=HL),
                ADD,
            )

        # --- Q projection: qt[p(2h x 64dh), pair, seq] ---
        wq = wT_tile(WqT, "wq")
        qblk = load_blocks(queryT, "qb")
        for j in range(NPAIR):
            for sc in range(4):
                ps = acc.tile([P, 512], f32, tag="acc")
                for kt in range(8):
                    nc.tensor.matmul(
                        out=ps[:],
                        lhsT=wq[:, kt, j * P:(j + 1) * P],
                        rhs=qblk[kt][:, sc * 512:(sc + 1) * 512],
                        start=(kt == 0), stop=(kt == 7),
                    )
                nc.vector.tensor_scalar_add(
                    qt[:, j, sc * 512:(sc + 1) * 512], ps[:], bq_sb[:, j:j + 1]
                )

        # --- attention: per (pair, query-chunk): QK^T -> exp -> PV ---
        for j in range(NPAIR):
            for qc in range(NQC):
                r = j * NQC + qc
                pt = [
                    ptp.tile([P, NKT, QC], bf16, tag="pt", name=f"pt{h}")
                    for h in range(2)
                ]
                for g in range(4):
                    ett = [
                        et.tile([P, 1024], f32, tag="et", name=f"et{h}")
                        for h in range(2)
                    ]
                    for t_i in range(4):
                        kti = g * 4 + t_i
                        for h in range(2):
                            nc.tensor.matmul(
                                out=ett[h][:, t_i * QC:(t_i + 1) * QC],
                                lhsT=kT[h * 64:(h + 1) * 64, j, kti * P:(kti + 1) * P],
                                rhs=qt[h * 64:(h + 1) * 64, j, qc * QC:(qc + 1) * QC],
                                start=True, stop=True,
                                tile_position=(h * 64, 0),
                            )
                    for h in range(2):
                        nc.scalar.activation(
                            pt[h][:, g * 4:(g + 1) * 4, :],
                            ett[h][:].rearrange("p (t q) -> p t q", t=4),
                            EXP,
                            scale=SCALE,
                        )
                stg = stgp.tile([P, 2, QC], f32, tag="dnst")
                for h in range(2):
                    pvt = acc.tile([P, 512], f32, tag="acc", name=f"pv{h}")
                    for kti in range(NKT):
                        nc.tensor.matmul(
                            out=pvt[0:DH + 1, 0:QC],
                            lhsT=v_pad[:, kti, 2 * j + h, 0:DH + 1],
                            rhs=pt[h][:, kti, :],
                            start=(kti == 0), stop=(kti == NKT - 1),
                        )
                    nc.vector.tensor_copy(
                        out=attn[h * 64:(h + 1) * 64, j, qc * QC:(qc + 1) * QC],
                        in_=pvt[0:DH, 0:QC],
                    )
                    nc.vector.tensor_copy(
                        out=stg[64:65, h, :],
                        in_=pvt[DH:DH + 1, 0:QC],
                    )
                nc.sync.dma_start(dn[r:r + 1, :], stg[64:65, :, :])

        # --- normalization: one batched reciprocal, DMA-broadcast, multiply ---
        attn2 = persist.tile([P, NPAIR, S], f16, tag="qa", name="attn2")
        rc_d = dram.tile([NPAIR * NQC * 2, QC], f32, tag="rc_d")
        nc.vector.reciprocal(rc[:], dn[:])
        nc.sync.dma_start(
            rc_d[:].rearrange("(r h) q -> r (h q)", h=2), rc[:]
        )
        for j in range(NPAIR):
            for qc in range(NQC):
                r = j * NQC + qc
                bc_t = bcp.tile([P, QC], f32, tag="bc")
                nc.sync.dma_start(
                    bc_t[0:64, :], rc_d[2 * r:2 * r + 1, :].to_broadcast([64, QC])
                )
                nc.sync.dma_start(
                    bc_t[64:P, :], rc_d[2 * r + 1:2 * r + 2, :].to_broadcast([64, QC])
                )
                nc.vector.tensor_tensor(
                    attn2[:, j, qc * QC:(qc + 1) * QC],
                    attn[:, j, qc * QC:(qc + 1) * QC],
                    bc_t[:],
                    MULT,
                )

        # --- output projection (partial over local 512 dims; bo/2 added) ---
        wo = wpool.tile([P, NPAIR, D], f16, tag="w", name="wo")
        nc.sync.dma_start(wo[:], WoT[:].rearrange("(c p) d -> p c d", p=P))
        for dt in range(8):
            for sc in range(4):
                ps = acc.tile([P, 512], f32, tag="acc")
                for ct in range(NPAIR):
                    nc.tensor.matmul(
                        out=ps[:],
                        lhsT=wo[:, ct, dt * P:(dt + 1) * P],
                        rhs=attn2[:, ct, sc * 512:(sc + 1) * 512],
                        start=(ct == 0), stop=(ct == NPAIR - 1),
                    )
                o_t = ost.tile([P, 512], f32, tag="ost")
                nc.vector.tensor_scalar_add(o_t[:], ps[:], bo_sb[:, dt:dt + 1])
                nc.sync.dma_start(
                    outT[dt * P:(dt + 1) * P, sc * 512:(sc + 1) * 512], o_t[:]
                )

    if not nc.is_finalized():
        nc.finalize()
    return nc


def get_nc():
    if "nc" not in _CACHE:
        _CACHE["nc"] = _build_nc()
    return _CACHE["nc"]


def make_in_maps(inputs):
    f16 = np.float16
    q = np.asarray(inputs["query"], np.float32)
    k = np.asarray(inputs["key"], np.float32)
    v = np.asarray(inputs["value"], np.float32)
    Wq = np.asarray(inputs["Wq"], np.float32)
    Wk = np.asarray(inputs["Wk"], np.float32)
    Wv = np.asarray(inputs["Wv"], np.float32)
    Wo = np.asarray(inputs["Wo"], np.float32)
    bq = np.asarray(inputs["bq"], np.float32)
    bk = np.asarray(inputs["bk"], np.float32)
    bv = np.asarray(inputs["bv"], np.float32)
    bo_half = np.asarray(inputs["bo"], np.float32) * 0.5

    qT = [np.ascontiguousarray(q[b].T.astype(f16)) for b in range(B)]
    kTl = [np.ascontiguousarray(k[b].T.astype(f16)) for b in range(B)]
    vT = [np.ascontiguousarray(v[b].T.astype(f16)) for b in range(B)]
    WqTs = [np.ascontiguousarray(Wq.T[:, i * DL:(i + 1) * DL].astype(f16)) for i in range(2)]
    WkTs = [np.ascontiguousarray(Wk.T[:, i * DL:(i + 1) * DL].astype(f16)) for i in range(2)]
    WvTs = [np.ascontiguousarray(Wv.T[:, i * DL:(i + 1) * DL].astype(f16)) for i in range(2)]
    WoTs = [np.ascontiguousarray(Wo.T[i * DL:(i + 1) * DL, :].astype(f16)) for i in range(2)]
    bqs = [np.ascontiguousarray(bq[i * DL:(i + 1) * DL]) for i in range(2)]
    bks = [np.ascontiguousarray(bk[i * DL:(i + 1) * DL]) for i in range(2)]
    bvs = [np.ascontiguousarray(bv[i * DL:(i + 1) * DL]).reshape(1, DL) for i in range(2)]

    in_maps = []
    for c in range(8):
        b, hh = c // 2, c % 2
        in_maps.append({
            "queryT": qT[b], "keyT": kTl[b], "valueT": vT[b],
            "WqT": WqTs[hh], "WkT": WkTs[hh], "WvT": WvTs[hh], "WoT": WoTs[hh],
            "bq": bqs[hh], "bk": bks[hh], "bv": bvs[hh], "bo_half": bo_half,
        })
    return in_maps


def assemble(results):
    out = np.empty((B, S, D), np.float32)
    for b in range(B):
        out[b] = (results[2 * b]["outT"] + results[2 * b + 1]["outT"]).T
    return out


def _numpy_fallback(inputs):
    q = np.asarray(inputs["query"], np.float64)
    k = np.asarray(inputs["key"], np.float64)
    v = np.asarray(inputs["value"], np.float64)
    Wq, bq = np.asarray(inputs["Wq"], np.float64), np.asarray(inputs["bq"], np.float64)
    Wk, bk = np.asarray(inputs["Wk"], np.float64), np.asarray(inputs["bk"], np.float64)
    Wv, bv = np.asarray(inputs["Wv"], np.float64), np.asarray(inputs["bv"], np.float64)
    Wo, bo = np.asarray(inputs["Wo"], np.float64), np.asarray(inputs["bo"], np.float64)
    qp = (q @ Wq.T + bq).reshape(B, S, H, DH).transpose(0, 2, 1, 3)
    kp = (k @ Wk.T + bk).reshape(B, S, H, DH).transpose(0, 2, 1, 3)
    vp = (v @ Wv.T + bv).reshape(B, S, H, DH).transpose(0, 2, 1, 3)
    e = np.einsum("bhqd,bhkd->bhqk", qp, kp) * SCALE
    mask = np.asarray(inputs["mask"])
    kpm = np.asarray(inputs["key_padding_mask"])
    e = np.where(mask == 0, -np.inf, e)
    e = np.where(kpm[:, None, None, :] == 0, -np.inf, e)
    e -= e.max(axis=-1, keepdims=True)
    p = np.exp(e)
    p /= p.sum(axis=-1, keepdims=True)
    o = np.einsum("bhqk,bhkd->bhqd", p, vp).transpose(0, 2, 1, 3).reshape(B, S, D)
    return (o @ Wo.T + bo).astype(np.float32)


def kernel(**inputs):
    mask = np.asarray(inputs["mask"])
    kpm = np.asarray(inputs["key_padding_mask"])
    if not (mask.all() and kpm.all()):
        return _numpy_fallback(inputs)
    from concourse.bass_utils import run_bass_kernel_spmd

    nc = get_nc()
    in_maps = make_in_maps(inputs)
    res = run_bass_kernel_spmd(nc, in_maps, list(range(8)))
    return assemble(res.results)


# revision 14
# speedup vs baseline: 1.5771x; 1.0323x over previous
"""Trainium2 Bass kernel for CustomAttention (B=4, S=2048, D=1024, H=16).

Sharding: 8 cores = 4 batches x 2 head-halves (8 heads each). Each core
computes Q/K/V projections for its 512 head-dims, attention for its 8 heads
over all 2048 queries, and a partial out-projection (contraction over its 512
dims). Host sums the two partial outputs per batch; bo/2 is added on each core
so the host sum carries the full bias.

Performance structure:
  - All matmul operands 16-bit (fp16 where the value range allows, bf16 where
    exp magnitudes flow: pt, v_pad, unnormalized attention). Full-rate PE.
  - softmax scale folded into Wq/bq host-side.
  - K^T/Q^T/attention SBUF-resident; inputs staged in [128,512] chunks.
  - Pipelined emission: K proj -> Q proj (pair-major, so attention starts as
    soon as pair 0's Q lands) -> V proj -> attention (qc-outer, pair-inner)
    with per-qc deferred normalization and out-projection chunks inlined every
    other qc. ScalarE exp is the critical engine; everything else hides under.
  - exp in {6,6,4}-kti segments (1536-elem calls amortize ACT fixed overhead,
    3 PSUM banks each, double buffered). PV for both heads accumulates into
    ONE PSUM bank ([65, 2, 256]; sequential per-head chains), ones-column of
    v_pad gives the softmax denominators; one batched reciprocal per qc.
  - mask / key_padding_mask are all-ones for this problem's inputs => identity;
    a numpy fallback handles the (never-hit) general case.
"""

import math

import numpy as np

B, S, D = 4, 2048, 1024
H, DH = 16, 64       # global heads
HL = 8               # local heads per core
P = 128
NPAIR = HL // 2      # 4 local head pairs
NKT = S // P         # 16 key tiles
QC = 256             # query chunk for attention
NQC = S // QC        # 8
DL = 512             # local projection width (8 heads x 64)
SCALE = math.log(D) / math.sqrt(DH)
SEGS = [(0, 6), (6, 6), (12, 4)]  # kti segments for QK/exp

_CACHE = {}


def _build_nc():
    import concourse.bass as bass
    import concourse.bacc as bacc
    import concourse.mybir as mybir
    import concourse.tile as tile
    from contextlib import ExitStack

    f32 = mybir.dt.float32
    f16 = mybir.dt.float16
    bf16 = mybir.dt.bfloat16
    EXP = mybir.ActivationFunctionType.Exp
    ADD = mybir.AluOpType.add
    MULT = mybir.AluOpType.mult

    nc = bacc.Bacc("TRN2", target_bir_lowering=False, debug=False, num_devices=8)

    queryT = nc.declare_dram_parameter("queryT", [D, S], f16, isOutput=False)
    keyT = nc.declare_dram_parameter("keyT", [D, S], f16, isOutput=False)
    valueT = nc.declare_dram_parameter("valueT", [D, S], f16, isOutput=False)
    WqT = nc.declare_dram_parameter("WqT", [D, DL], f16, isOutput=False)
    WkT = nc.declare_dram_parameter("WkT", [D, DL], f16, isOutput=False)
    WvT = nc.declare_dram_parameter("WvT", [D, DL], f16, isOutput=False)
    WoT = nc.declare_dram_parameter("WoT", [DL, D], bf16, isOutput=False)
    bq_d = nc.declare_dram_parameter("bq", [DL], f32, isOutput=False)
    bk_d = nc.declare_dram_parameter("bk", [DL], f32, isOutput=False)
    bv_d = nc.declare_dram_parameter("bv", [1, DL], f32, isOutput=False)
    bo_d = nc.declare_dram_parameter("bo_half", [D], f32, isOutput=False)
    outT = nc.declare_dram_parameter("outT", [D, S], f32, isOutput=True)

    with ExitStack() as ctx:
        tc = ctx.enter_context(tile.TileContext(nc))
        persist = ctx.enter_context(tc.tile_pool(name="persist", bufs=1))
        wpool = ctx.enter_context(tc.tile_pool(name="wpool", bufs=1))
        in512 = ctx.enter_context(tc.tile_pool(name="in512", bufs=20))
        ptp = ctx.enter_context(tc.tile_pool(name="ptp", bufs=8))
        bcp = ctx.enter_context(tc.tile_pool(name="bcp", bufs=4))
        stgp = ctx.enter_context(tc.tile_pool(name="stgp", bufs=2))
        ost = ctx.enter_context(tc.tile_pool(name="ost", bufs=2))
        dnp = ctx.enter_context(tc.tile_pool(name="dnp", bufs=2))
        rcp = ctx.enter_context(tc.tile_pool(name="rcp", bufs=2))
        et = ctx.enter_context(tc.tile_pool(name="et", bufs=2, space="PSUM"))
        pvp = ctx.enter_context(tc.tile_pool(name="pvp", bufs=1, space="PSUM"))
        acc = ctx.enter_context(tc.tile_pool(name="acc", bufs=1, space="PSUM"))
        dram = ctx.enter_context(tc.tile_pool(name="dram", bufs=2, space="DRAM"))

        kT = [
            persist.tile([P, S], f16, tag=f"kT{j}", name=f"kT{j}")
            for j in range(NPAIR)
        ]
        qt = [
            persist.tile([P, S], f16, tag=f"qt{j}", name=f"qt{j}")
            for j in range(NPAIR)
        ]
        attn = [
            persist.tile([P, S], bf16, tag=f"at{j}", name=f"at{j}")
            for j in range(NPAIR)
        ]
        v_pad = persist.tile([P, NKT, HL, DH + 1], bf16, tag="v_pad")
        bq_sb = persist.tile([P, NPAIR], f32, tag="bq")
        bk_sb = persist.tile([P, NPAIR], f32, tag="bk")
        bo_sb = persist.tile([P, 8], f32, tag="bo")
        bv_bc = persist.tile([P, DL], f32, tag="bv_bc")

        # --- setup ---
        nc.sync.dma_start(bq_sb[:], bq_d.rearrange("(o p) -> p o", p=P))
        nc.sync.dma_start(bk_sb[:], bk_d.rearrange("(o p) -> p o", p=P))
        nc.sync.dma_start(bo_sb[:], bo_d.rearrange("(o p) -> p o", p=P))
        nc.sync.dma_start(bv_bc[:], bv_d[:].to_broadcast([P, DL]))
        nc.vector.memset(v_pad[:], 1.0)

        wk = wpool.tile([P, 8, DL], f16, tag="w", name="wk")
        nc.sync.dma_start(wk[:], WkT[:].rearrange("(k p) c -> p k c", p=P))
        wq = wpool.tile([P, 8, DL], f16, tag="w2", name="wq")
        nc.sync.dma_start(wq[:], WqT[:].rearrange("(k p) c -> p k c", p=P))
        wv = wpool.tile([P, 8, DL], f16, tag="w3", name="wv")
        nc.sync.dma_start(wv[:], WvT[:].rearrange("(k p) c -> p k c", p=P))
        wo = wpool.tile([P, NPAIR, D], bf16, tag="w4", name="wo")
        nc.sync.dma_start(wo[:], WoT[:].rearrange("(c p) d -> p c d", p=P))

        def chunk(srcT, kt, sc, name):
            t = in512.tile([P, 512], f16, tag="in", name=name)
            nc.sync.dma_start(
                t[:], srcT[kt * P:(kt + 1) * P, sc * 512:(sc + 1) * 512]
            )
            return t

        def proj_psum(i):
            # alternate between the two 1-bank pools for 2-deep pipelining
            pool = acc if i % 2 == 0 else pvp
            return pool.tile([P, 512], f32, tag="acc" if i % 2 == 0 else "pv", name=f"pp{i}")

        # --- K projection: kT[j][p(2h x 64dh), seq], sc-major ---
        pi = 0
        for sc in range(4):
            kc = [chunk(keyT, kt, sc, f"kc{kt}_{sc}") for kt in range(8)]
            for j in range(NPAIR):
                ps = proj_psum(pi); pi += 1
                for kt in range(8):
                    nc.tensor.matmul(
                        out=ps[:],
                        lhsT=wk[:, kt, j * P:(j + 1) * P],
                        rhs=kc[kt][:],
                        start=(kt == 0), stop=(kt == 7),
                    )
                nc.vector.tensor_scalar_add(
                    kT[j][:, sc * 512:(sc + 1) * 512], ps[:], bk_sb[:, j:j + 1]
                )

        # --- Q projection: sc-major; attention qc reads only its 256-col slice,
        # so all pairs' first chunks land after the first sc iteration ---
        for sc in range(4):
            qc_ = [chunk(queryT, kt, sc, f"qc{kt}_{sc}") for kt in range(8)]
            for j in range(NPAIR):
                ps = proj_psum(pi); pi += 1
                for kt in range(8):
                    nc.tensor.matmul(
                        out=ps[:],
                        lhsT=wq[:, kt, j * P:(j + 1) * P],
                        rhs=qc_[kt][:],
                        start=(kt == 0), stop=(kt == 7),
                    )
                nc.vector.tensor_scalar_add(
                    qt[j][:, sc * 512:(sc + 1) * 512], ps[:], bq_sb[:, j:j + 1]
                )

        # --- V projection: v_pad[p(key), kti, head, dh + ones col] ---
        for stg4 in range(4):
            vc = [chunk(valueT, kt, stg4, f"vc{kt}_{stg4}") for kt in range(8)]
            for stl in range(4):
                st = stg4 * 4 + stl
                ps = proj_psum(pi); pi += 1
                for kt in range(8):
                    nc.tensor.matmul(
                        out=ps[:],
                        lhsT=vc[kt][:, stl * P:(stl + 1) * P],
                        rhs=wv[:, kt, :],
                        start=(kt == 0), stop=(kt == 7),
                    )
                nc.vector.tensor_tensor(
                    v_pad[:, st, :, 0:DH],
                    ps[:].rearrange("p (h d) -> p h d", h=HL),
                    bv_bc[:].rearrange("p (h d) -> p h d", h=HL),
                    ADD,
                )

        # --- attention: qc-outer, pair-inner; deferred normalization ---
        for qc in range(NQC):
            dnq = dnp.tile([NPAIR, 2, QC], f32, tag="dn")
            for j in range(NPAIR):
                pt = [
                    ptp.tile([P, NKT, QC], bf16, tag="pt", name=f"pt{h}")
                    for h in range(2)
                ]
                for sb, sl in SEGS:
                    ett = [
                        et.tile([P, 6, QC], f32, tag="et", name=f"et{h}")
                        for h in range(2)
                    ]
                    for t_i in range(sl):
                        kti = sb + t_i
                        for h in range(2):
                            nc.tensor.matmul(
                                out=ett[h][:, t_i, :],
                                lhsT=kT[j][h * 64:(h + 1) * 64, kti * P:(kti + 1) * P],
                                rhs=qt[j][h * 64:(h + 1) * 64, qc * QC:(qc + 1) * QC],
                                start=True, stop=True,
                                tile_position=(h * 64, 0),
                            )
                    for h in range(2):
                        nc.scalar.activation(
                            pt[h][:, sb:sb + sl, :], ett[h][:, 0:sl, :], EXP
                        )
                pvt = pvp.tile([DH + 1, 2, QC], f32, tag="pv")
                for h in range(2):
                    for kti in range(NKT):
                        nc.tensor.matmul(
                            out=pvt[0:DH + 1, h, :],
                            lhsT=v_pad[:, kti, 2 * j + h, 0:DH + 1],
                            rhs=pt[h][:, kti, :],
                            start=(kti == 0), stop=(kti == NKT - 1),
                        )
                stg = stgp.tile([P, 2, QC], f32, tag="dnst")
                for h in range(2):
                    nc.vector.tensor_copy(
                        out=attn[j][h * 64:(h + 1) * 64, qc * QC:(qc + 1) * QC],
                        in_=pvt[0:DH, h, :],
                    )
                    nc.vector.tensor_copy(
                        out=stg[64:65, h, :], in_=pvt[DH:DH + 1, h, :]
                    )
                nc.sync.dma_start(dnq[j:j + 1, :, :], stg[64:65, :, :])

            rcq = rcp.tile([NPAIR, 2, QC], f32, tag="rc")
            nc.vector.reciprocal(rcq[:], dnq[:])
            rcd = dram.tile([NPAIR * 2, QC], f32, tag="rcd")
            nc.sync.dma_start(rcd[:].rearrange("(j h) q -> j h q", h=2), rcq[:])
            for j in range(NPAIR):
                bc_t = bcp.tile([P, QC], f32, tag="bc")
                nc.sync.dma_start(
                    bc_t[0:64, :],
                    rcd[2 * j:2 * j + 1, :].to_broadcast([64, QC]),
                )
                nc.sync.dma_start(
                    bc_t[64:P, :],
                    rcd[2 * j + 1:2 * j + 2, :].to_broadcast([64, QC]),
                )
                nc.vector.tensor_tensor(
                    attn[j][:, qc * QC:(qc + 1) * QC],
                    attn[j][:, qc * QC:(qc + 1) * QC],
                    bc_t[:],
                    MULT,
                )

            # --- inline partial out-projection every other qc ---
            if qc % 2 == 1:
                sc = qc // 2
                for dt in range(8):
                    ps = acc.tile([P, 512], f32, tag="acc")
                    for ct in range(NPAIR):
                        nc.tensor.matmul(
                            out=ps[:],
                            lhsT=wo[:, ct, dt * P:(dt + 1) * P],
                            rhs=attn[ct][:, sc * 512:(sc + 1) * 512],
                            start=(ct == 0), stop=(ct == NPAIR - 1),
                        )
                    o_t = ost.tile([P, 512], f32, tag="ost")
                    nc.vector.tensor_scalar_add(o_t[:], ps[:], bo_sb[:, dt:dt + 1])
                    nc.sync.dma_start(
                        outT[dt * P:(dt + 1) * P, sc * 512:(sc + 1) * 512], o_t[:]
                    )

    if not nc.is_finalized():
        nc.finalize()
    return nc


def get_nc():
    if "nc" not in _CACHE:
        _CACHE["nc"] = _build_nc()
    return _CACHE["nc"]


def make_in_maps(inputs):
    f16 = np.float16
    import ml_dtypes

    bf16 = ml_dtypes.bfloat16
    q = np.asarray(inputs["query"], np.float32)
    k = np.asarray(inputs["key"], np.float32)
    v = np.asarray(inputs["value"], np.float32)
    Wq = np.asarray(inputs["Wq"], np.float32) * SCALE  # fold softmax scale
    Wk = np.asarray(inputs["Wk"], np.float32)
    Wv = np.asarray(inputs["Wv"], np.float32)
    Wo = np.asarray(inputs["Wo"], np.float32)
    bq = np.asarray(inputs["bq"], np.float32) * SCALE
    bk = np.asarray(inputs["bk"], np.float32)
    bv = np.asarray(inputs["bv"], np.float32)
    bo_half = np.asarray(inputs["bo"], np.float32) * 0.5

    qT = [np.ascontiguousarray(q[b].T.astype(f16)) for b in range(B)]
    kTl = [np.ascontiguousarray(k[b].T.astype(f16)) for b in range(B)]
    vT = [np.ascontiguousarray(v[b].T.astype(f16)) for b in range(B)]
    WqTs = [np.ascontiguousarray(Wq.T[:, i * DL:(i + 1) * DL].astype(f16)) for i in range(2)]
    WkTs = [np.ascontiguousarray(Wk.T[:, i * DL:(i + 1) * DL].astype(f16)) for i in range(2)]
    WvTs = [np.ascontiguousarray(Wv.T[:, i * DL:(i + 1) * DL].astype(f16)) for i in range(2)]
    WoTs = [np.ascontiguousarray(Wo.T[i * DL:(i + 1) * DL, :].astype(bf16)) for i in range(2)]
    bqs = [np.ascontiguousarray(bq[i * DL:(i + 1) * DL]) for i in range(2)]
    bks = [np.ascontiguousarray(bk[i * DL:(i + 1) * DL]) for i in range(2)]
    bvs = [np.ascontiguousarray(bv[i * DL:(i + 1) * DL]).reshape(1, DL) for i in range(2)]

    in_maps = []
    for c in range(8):
        b, hh = c // 2, c % 2
        in_maps.append({
            "queryT": qT[b], "keyT": kTl[b], "valueT": vT[b],
            "WqT": WqTs[hh], "WkT": WkTs[hh], "WvT": WvTs[hh], "WoT": WoTs[hh],
            "bq": bqs[hh], "bk": bks[hh], "bv": bvs[hh], "bo_half": bo_half,
        })
    return in_maps


def assemble(results):
    out = np.empty((B, S, D), np.float32)
    for b in range(B):
        out[b] = (results[2 * b]["outT"] + results[2 * b + 1]["outT"]).T
    return out


def _numpy_fallback(inputs):
    q = np.asarray(inputs["query"], np.float64)
    k = np.asarray(inputs["key"], np.float64)
    v = np.asarray(inputs["value"], np.float64)
    Wq, bq = np.asarray(inputs["Wq"], np.float64), np.asarray(inputs["bq"], np.float64)
    Wk, bk = np.asarray(inputs["Wk"], np.float64), np.asarray(inputs["bk"], np.float64)
    Wv, bv = np.asarray(inputs["Wv"], np.float64), np.asarray(inputs["bv"], np.float64)
    Wo, bo = np.asarray(inputs["Wo"], np.float64), np.asarray(inputs["bo"], np.float64)
    qp = (q @ Wq.T + bq).reshape(B, S, H, DH).transpose(0, 2, 1, 3)
    kp = (k @ Wk.T + bk).reshape(B, S, H, DH).transpose(0, 2, 1, 3)
    vp = (v @ Wv.T + bv).reshape(B, S, H, DH).transpose(0, 2, 1, 3)
    e = np.einsum("bhqd,bhkd->bhqk", qp, kp) * SCALE
    mask = np.asarray(inputs["mask"])
    kpm = np.asarray(inputs["key_padding_mask"])
    e = np.where(mask == 0, -np.inf, e)
    e = np.where(kpm[:, None, None, :] == 0, -np.inf, e)
    e -= e.max(axis=-1, keepdims=True)
    p = np.exp(e)
    p /= p.sum(axis=-1, keepdims=True)
    o = np.einsum("bhqk,bhkd->bhqd", p, vp).transpose(0, 2, 1, 3).reshape(B, S, D)
    return (o @ Wo.T + bo).astype(np.float32)


def kernel(**inputs):
    mask = np.asarray(inputs["mask"])
    kpm = np.asarray(inputs["key_padding_mask"])
    if not (mask.all() and kpm.all()):
        return _numpy_fallback(inputs)
    from concourse.bass_utils import run_bass_kernel_spmd

    nc = get_nc()
    in_maps = make_in_maps(inputs)
    res = run_bass_kernel_spmd(nc, in_maps, list(range(8)))
    return assemble(res.results)


# revision 15
# speedup vs baseline: 1.8234x; 1.1562x over previous
"""Trainium2 Bass kernel for CustomAttention (B=4, S=2048, D=1024, H=16).

Sharding: 8 cores = 4 batches x 2 head-halves (8 heads each). Each core
computes Q/K/V projections for its 512 head-dims, attention for its 8 heads
over all 2048 queries, and a partial out-projection (contraction over its 512
dims). Host sums the two partial outputs per batch; bo/2 is added on each core
so the host sum carries the full bias.

Performance structure:
  - All matmul operands 16-bit (fp16 where the value range allows, bf16 where
    exp magnitudes flow: pt, v_pad, unnormalized attention). Full-rate PE.
  - softmax scale folded into Wq/bq host-side.
  - K^T/Q^T/attention SBUF-resident; inputs staged in [128,512] chunks.
  - Pipelined emission: K proj -> Q proj (pair-major, so attention starts as
    soon as pair 0's Q lands) -> V proj -> attention (qc-outer, pair-inner)
    with per-qc deferred normalization and out-projection chunks inlined every
    other qc. ScalarE exp is the critical engine; everything else hides under.
  - exp in {6,6,4}-kti segments (1536-elem calls amortize ACT fixed overhead,
    3 PSUM banks each, double buffered). PV for both heads accumulates into
    ONE PSUM bank ([65, 2, 256]; sequential per-head chains), ones-column of
    v_pad gives the softmax denominators; one batched reciprocal per qc.
  - mask / key_padding_mask are all-ones for this problem's inputs => identity;
    a numpy fallback handles the (never-hit) general case.
"""

import math

import numpy as np

B, S, D = 4, 2048, 1024
H, DH = 16, 64       # global heads
HL = 8               # local heads per core
P = 128
NPAIR = HL // 2      # 4 local head pairs
NKT = S // P         # 16 key tiles
QC = 256             # query chunk for attention
NQC = S // QC        # 8
DL = 512             # local projection width (8 heads x 64)
SCALE = math.log(D) / math.sqrt(DH)
SEGS = [(0, 6), (6, 6), (12, 4)]  # kti segments for QK/exp

_CACHE = {}


def _build_nc():
    import concourse.bass as bass
    import concourse.bacc as bacc
    import concourse.mybir as mybir
    import concourse.tile as tile
    from contextlib import ExitStack

    f32 = mybir.dt.float32
    f16 = mybir.dt.float16
    bf16 = mybir.dt.bfloat16
    EXP = mybir.ActivationFunctionType.Exp
    ADD = mybir.AluOpType.add
    MULT = mybir.AluOpType.mult

    nc = bacc.Bacc("TRN2", target_bir_lowering=False, debug=False, num_devices=8)

    queryT = nc.declare_dram_parameter("queryT", [D, S], f16, isOutput=False)
    keyT = nc.declare_dram_parameter("keyT", [D, S], f16, isOutput=False)
    valueT = nc.declare_dram_parameter("valueT", [D, S], f16, isOutput=False)
    WqT = nc.declare_dram_parameter("WqT", [D, DL], f16, isOutput=False)
    WkT = nc.declare_dram_parameter("WkT", [D, DL], f16, isOutput=False)
    WvT = nc.declare_dram_parameter("WvT", [D, DL], f16, isOutput=False)
    WoT = nc.declare_dram_parameter("WoT", [DL, D], bf16, isOutput=False)
    bq_d = nc.declare_dram_parameter("bq", [DL], f32, isOutput=False)
    bk_d = nc.declare_dram_parameter("bk", [DL], f32, isOutput=False)
    bv_d = nc.declare_dram_parameter("bv", [1, DL], f32, isOutput=False)
    bo_d = nc.declare_dram_parameter("bo_half", [D], f32, isOutput=False)
    outT = nc.declare_dram_parameter("outT", [D, S], f32, isOutput=True)

    with ExitStack() as ctx:
        tc = ctx.enter_context(tile.TileContext(nc))
        persist = ctx.enter_context(tc.tile_pool(name="persist", bufs=1))
        wpool = ctx.enter_context(tc.tile_pool(name="wpool", bufs=1))
        in512 = ctx.enter_context(tc.tile_pool(name="in512", bufs=20))
        ptp = ctx.enter_context(tc.tile_pool(name="ptp", bufs=8))
        bcp = ctx.enter_context(tc.tile_pool(name="bcp", bufs=4))
        stgp = ctx.enter_context(tc.tile_pool(name="stgp", bufs=2))
        ost = ctx.enter_context(tc.tile_pool(name="ost", bufs=2))
        dnp = ctx.enter_context(tc.tile_pool(name="dnp", bufs=2))
        rcp = ctx.enter_context(tc.tile_pool(name="rcp", bufs=2))
        et = ctx.enter_context(tc.tile_pool(name="et", bufs=2, space="PSUM"))
        pvp = ctx.enter_context(tc.tile_pool(name="pvp", bufs=1, space="PSUM"))
        acc = ctx.enter_context(tc.tile_pool(name="acc", bufs=1, space="PSUM"))
        dram = ctx.enter_context(tc.tile_pool(name="dram", bufs=2, space="DRAM"))

        kT = [
            persist.tile([P, S], f16, tag=f"kT{j}", name=f"kT{j}")
            for j in range(NPAIR)
        ]
        qt = [
            persist.tile([P, S], f16, tag=f"qt{j}", name=f"qt{j}")
            for j in range(NPAIR)
        ]
        attn = [
            persist.tile([P, S], bf16, tag=f"at{j}", name=f"at{j}")
            for j in range(NPAIR)
        ]
        v_pad = persist.tile([P, NKT, HL, DH + 1], bf16, tag="v_pad")
        bq_sb = persist.tile([P, NPAIR], f32, tag="bq")
        bk_sb = persist.tile([P, NPAIR], f32, tag="bk")
        bo_sb = persist.tile([P, 8], f32, tag="bo")
        bv_bc = persist.tile([P, DL], f32, tag="bv_bc")

        # --- setup ---
        nc.sync.dma_start(bq_sb[:], bq_d.rearrange("(o p) -> p o", p=P))
        nc.sync.dma_start(bk_sb[:], bk_d.rearrange("(o p) -> p o", p=P))
        nc.sync.dma_start(bo_sb[:], bo_d.rearrange("(o p) -> p o", p=P))
        nc.sync.dma_start(bv_bc[:], bv_d[:].to_broadcast([P, DL]))
        nc.vector.memset(v_pad[:], 1.0)

        wk = wpool.tile([P, 8, DL], f16, tag="w", name="wk")
        nc.sync.dma_start(wk[:], WkT[:].rearrange("(k p) c -> p k c", p=P))
        wq = wpool.tile([P, 8, DL], f16, tag="w2", name="wq")
        nc.sync.dma_start(wq[:], WqT[:].rearrange("(k p) c -> p k c", p=P))
        wv = wpool.tile([P, 8, DL], f16, tag="w3", name="wv")
        nc.sync.dma_start(wv[:], WvT[:].rearrange("(k p) c -> p k c", p=P))
        wo = wpool.tile([P, NPAIR, D], bf16, tag="w4", name="wo")
        nc.sync.dma_start(wo[:], WoT[:].rearrange("(c p) d -> p c d", p=P))

        def chunk(srcT, kt, sc, name):
            t = in512.tile([P, 512], f16, tag="in", name=name)
            nc.sync.dma_start(
                t[:], srcT[kt * P:(kt + 1) * P, sc * 512:(sc + 1) * 512]
            )
            return t

        def proj_psum(i):
            # alternate between the two 1-bank pools for 2-deep pipelining
            pool = acc if i % 2 == 0 else pvp
            return pool.tile([P, 512], f32, tag="acc" if i % 2 == 0 else "pv", name=f"pp{i}")

        # --- K projection: kT[j][p(2h x 64dh), seq], sc-major ---
        pi = 0
        for sc in range(4):
            kc = [chunk(keyT, kt, sc, f"kc{kt}_{sc}") for kt in range(8)]
            for j in range(NPAIR):
                ps = proj_psum(pi); pi += 1
                for kt in range(8):
                    nc.tensor.matmul(
                        out=ps[:],
                        lhsT=wk[:, kt, j * P:(j + 1) * P],
                        rhs=kc[kt][:],
                        start=(kt == 0), stop=(kt == 7),
                    )
                nc.vector.tensor_scalar_add(
                    kT[j][:, sc * 512:(sc + 1) * 512], ps[:], bk_sb[:, j:j + 1]
                )

        # --- Q projection emitter (one sc chunk of all pairs at a time) ---
        def q_proj_sc(sc):
            nonlocal pi
            qc_ = [chunk(queryT, kt, sc, f"qc{kt}_{sc}") for kt in range(8)]
            for j in range(NPAIR):
                ps = proj_psum(pi); pi += 1
                for kt in range(8):
                    nc.tensor.matmul(
                        out=ps[:],
                        lhsT=wq[:, kt, j * P:(j + 1) * P],
                        rhs=qc_[kt][:],
                        start=(kt == 0), stop=(kt == 7),
                    )
                nc.vector.tensor_scalar_add(
                    qt[j][:, sc * 512:(sc + 1) * 512], ps[:], bq_sb[:, j:j + 1]
                )

        # --- V projection emitter: one pair-group (2 pairs = 256 dims) pass.
        # Split so PV of early pairs can start before all of V is projected. ---
        def v_proj_half(g):
            nonlocal pi
            for stg4 in range(4):
                vc = [
                    chunk(valueT, kt, stg4, f"vc{g}_{kt}_{stg4}")
                    for kt in range(8)
                ]
                for stl in range(4):
                    st = stg4 * 4 + stl
                    ps = proj_psum(pi); pi += 1
                    for kt in range(8):
                        nc.tensor.matmul(
                            out=ps[:, 0:256],
                            lhsT=vc[kt][:, stl * P:(stl + 1) * P],
                            rhs=wv[:, kt, g * 256:(g + 1) * 256],
                            start=(kt == 0), stop=(kt == 7),
                        )
                    nc.vector.tensor_tensor(
                        v_pad[:, st, g * 4:(g + 1) * 4, 0:DH],
                        ps[:, 0:256].rearrange("p (h d) -> p h d", h=4),
                        bv_bc[:, g * 256:(g + 1) * 256].rearrange(
                            "p (h d) -> p h d", h=4
                        ),
                        ADD,
                    )

        # --- attention emitters: QK+exp and (staggered) PV+normalization ---
        pt_of = {}
        dnq_of = {}

        def att_qk(qc, j):
            pt = [
                ptp.tile([P, NKT, QC], bf16, tag="pt", name=f"pt{qc}_{j}_{h}")
                for h in range(2)
            ]
            pt_of[(qc, j)] = pt
            for sb, sl in SEGS:
                ett = [
                    et.tile([P, 6, QC], f32, tag="et", name=f"et{h}")
                    for h in range(2)
                ]
                for t_i in range(sl):
                    kti = sb + t_i
                    for h in range(2):
                        nc.tensor.matmul(
                            out=ett[h][:, t_i, :],
                            lhsT=kT[j][h * 64:(h + 1) * 64, kti * P:(kti + 1) * P],
                            rhs=qt[j][h * 64:(h + 1) * 64, qc * QC:(qc + 1) * QC],
                            start=True, stop=True,
                            tile_position=(h * 64, 0),
                        )
                for h in range(2):
                    nc.scalar.activation(
                        pt[h][:, sb:sb + sl, :], ett[h][:, 0:sl, :], EXP
                    )

        def att_pv(qc, j):
            if j == 0:
                dnq_of[qc] = dnp.tile(
                    [NPAIR, 2, QC], f32, tag="dn", name=f"dn{qc}"
                )
            dnq = dnq_of[qc]
            pt = pt_of.pop((qc, j))
            pvt = pvp.tile([DH + 1, 2, QC], f32, tag="pv", name=f"pv{qc}_{j}")
            for h in range(2):
                for kti in range(NKT):
                    nc.tensor.matmul(
                        out=pvt[0:DH + 1, h, :],
                        lhsT=v_pad[:, kti, 2 * j + h, 0:DH + 1],
                        rhs=pt[h][:, kti, :],
                        start=(kti == 0), stop=(kti == NKT - 1),
                    )
            stg = stgp.tile([P, 2, QC], f32, tag="dnst")
            for h in range(2):
                nc.vector.tensor_copy(
                    out=attn[j][h * 64:(h + 1) * 64, qc * QC:(qc + 1) * QC],
                    in_=pvt[0:DH, h, :],
                )
                nc.vector.tensor_copy(
                    out=stg[64:65, h, :], in_=pvt[DH:DH + 1, h, :]
                )
            nc.sync.dma_start(dnq[j:j + 1, :, :], stg[64:65, :, :])
            if j == NPAIR - 1:
                norm(qc)
                if qc % 2 == 1:
                    o_proj(qc // 2)

        def norm(qc):
            dnq = dnq_of.pop(qc)
            rcq = rcp.tile([NPAIR, 2, QC], f32, tag="rc", name=f"rc{qc}")
            nc.vector.reciprocal(rcq[:], dnq[:])
            rcd = dram.tile([NPAIR * 2, QC], f32, tag="rcd", name=f"rcd{qc}")
            nc.sync.dma_start(rcd[:].rearrange("(j h) q -> j h q", h=2), rcq[:])
            for j in range(NPAIR):
                bc_t = bcp.tile([P, QC], f32, tag="bc")
                nc.sync.dma_start(
                    bc_t[0:64, :],
                    rcd[2 * j:2 * j + 1, :].to_broadcast([64, QC]),
                )
                nc.sync.dma_start(
                    bc_t[64:P, :],
                    rcd[2 * j + 1:2 * j + 2, :].to_broadcast([64, QC]),
                )
                nc.vector.tensor_tensor(
                    attn[j][:, qc * QC:(qc + 1) * QC],
                    attn[j][:, qc * QC:(qc + 1) * QC],
                    bc_t[:],
                    MULT,
                )

        def o_proj(sc):
            for dt in range(8):
                ps = acc.tile([P, 512], f32, tag="acc", name=f"op{sc}_{dt}")
                for ct in range(NPAIR):
                    nc.tensor.matmul(
                        out=ps[:],
                        lhsT=wo[:, ct, dt * P:(dt + 1) * P],
                        rhs=attn[ct][:, sc * 512:(sc + 1) * 512],
                        start=(ct == 0), stop=(ct == NPAIR - 1),
                    )
                o_t = ost.tile([P, 512], f32, tag="ost")
                nc.vector.tensor_scalar_add(o_t[:], ps[:], bo_sb[:, dt:dt + 1])
                nc.sync.dma_start(
                    outT[dt * P:(dt + 1) * P, sc * 512:(sc + 1) * 512], o_t[:]
                )

        # --- pipelined emission: PV lags QK by 3 units; Q sc1-3 and the two
        # V passes interleave into the attention stream (PE slack under ACT) ---
        units = [(qc, j) for qc in range(NQC) for j in range(NPAIR)]
        LAG = 3
        q_proj_sc(0)
        fills = {
            3: lambda: v_proj_half(0),   # before pv(u0)=qc0,j0 (pairs 0-1)
            4: lambda: q_proj_sc(1),
            5: lambda: v_proj_half(1),   # before pv(u2)=qc0,j2 (pairs 2-3)
            9: lambda: q_proj_sc(2),
            13: lambda: q_proj_sc(3),
        }
        for u in range(len(units) + LAG):
            if u in fills:
                fills[u]()
            if u < len(units):
                att_qk(*units[u])
            if u >= LAG:
                att_pv(*units[u - LAG])

    if not nc.is_finalized():
        nc.finalize()
    return nc


def get_nc():
    if "nc" not in _CACHE:
        _CACHE["nc"] = _build_nc()
    return _CACHE["nc"]


def make_in_maps(inputs):
    f16 = np.float16
    import ml_dtypes

    bf16 = ml_dtypes.bfloat16
    q = np.asarray(inputs["query"], np.float32)
    k = np.asarray(inputs["key"], np.float32)
    v = np.asarray(inputs["value"], np.float32)
    Wq = np.asarray(inputs["Wq"], np.float32) * SCALE  # fold softmax scale
    Wk = np.asarray(inputs["Wk"], np.float32)
    Wv = np.asarray(inputs["Wv"], np.float32)
    Wo = np.asarray(inputs["Wo"], np.float32)
    bq = np.asarray(inputs["bq"], np.float32) * SCALE
    bk = np.asarray(inputs["bk"], np.float32)
    bv = np.asarray(inputs["bv"], np.float32)
    bo_half = np.asarray(inputs["bo"], np.float32) * 0.5

    qT = [np.ascontiguousarray(q[b].T.astype(f16)) for b in range(B)]
    kTl = [np.ascontiguousarray(k[b].T.astype(f16)) for b in range(B)]
    vT = [np.ascontiguousarray(v[b].T.astype(f16)) for b in range(B)]
    WqTs = [np.ascontiguousarray(Wq.T[:, i * DL:(i + 1) * DL].astype(f16)) for i in range(2)]
    WkTs = [np.ascontiguousarray(Wk.T[:, i * DL:(i + 1) * DL].astype(f16)) for i in range(2)]
    WvTs = [np.ascontiguousarray(Wv.T[:, i * DL:(i + 1) * DL].astype(f16)) for i in range(2)]
    WoTs = [np.ascontiguousarray(Wo.T[i * DL:(i + 1) * DL, :].astype(bf16)) for i in range(2)]
    bqs = [np.ascontiguousarray(bq[i * DL:(i + 1) * DL]) for i in range(2)]
    bks = [np.ascontiguousarray(bk[i * DL:(i + 1) * DL]) for i in range(2)]
    bvs = [np.ascontiguousarray(bv[i * DL:(i + 1) * DL]).reshape(1, DL) for i in range(2)]

    in_maps = []
    for c in range(8):
        b, hh = c // 2, c % 2
        in_maps.append({
            "queryT": qT[b], "keyT": kTl[b], "valueT": vT[b],
            "WqT": WqTs[hh], "WkT": WkTs[hh], "WvT": WvTs[hh], "WoT": WoTs[hh],
            "bq": bqs[hh], "bk": bks[hh], "bv": bvs[hh], "bo_half": bo_half,
        })
    return in_maps


def assemble(results):
    out = np.empty((B, S, D), np.float32)
    for b in range(B):
        out[b] = (results[2 * b]["outT"] + results[2 * b + 1]["outT"]).T
    return out


def _numpy_fallback(inputs):
    q = np.asarray(inputs["query"], np.float64)
    k = np.asarray(inputs["key"], np.float64)
    v = np.asarray(inputs["value"], np.float64)
    Wq, bq = np.asarray(inputs["Wq"], np.float64), np.asarray(inputs["bq"], np.float64)
    Wk, bk = np.asarray(inputs["Wk"], np.float64), np.asarray(inputs["bk"], np.float64)
    Wv, bv = np.asarray(inputs["Wv"], np.float64), np.asarray(inputs["bv"], np.float64)
    Wo, bo = np.asarray(inputs["Wo"], np.float64), np.asarray(inputs["bo"], np.float64)
    qp = (q @ Wq.T + bq).reshape(B, S, H, DH).transpose(0, 2, 1, 3)
    kp = (k @ Wk.T + bk).reshape(B, S, H, DH).transpose(0, 2, 1, 3)
    vp = (v @ Wv.T + bv).reshape(B, S, H, DH).transpose(0, 2, 1, 3)
    e = np.einsum("bhqd,bhkd->bhqk", qp, kp) * SCALE
    mask = np.asarray(inputs["mask"])
    kpm = np.asarray(inputs["key_padding_mask"])
    e = np.where(mask == 0, -np.inf, e)
    e = np.where(kpm[:, None, None, :] == 0, -np.inf, e)
    e -= e.max(axis=-1, keepdims=True)
    p = np.exp(e)
    p /= p.sum(axis=-1, keepdims=True)
    o = np.einsum("bhqk,bhkd->bhqd", p, vp).transpose(0, 2, 1, 3).reshape(B, S, D)
    return (o @ Wo.T + bo).astype(np.float32)


def kernel(**inputs):
    mask = np.asarray(inputs["mask"])
    kpm = np.asarray(inputs["key_padding_mask"])
    if not (mask.all() and kpm.all()):
        return _numpy_fallback(inputs)
    from concourse.bass_utils import run_bass_kernel_spmd

    nc = get_nc()
    in_maps = make_in_maps(inputs)
    res = run_bass_kernel_spmd(nc, in_maps, list(range(8)))
    return assemble(res.results)
